# revision 39
# baseline (speedup 1.0000x reference)
"""AttentionBlock Trainium2 Bass kernel.

Problem: x[16,512,32,32] -> qkv proj -> 8-head attention (dk=64) over the
1024 spatial positions -> out proj + residual -> [16,512,32,32].

Sharding: data-parallel over batch; 2 images per core on 8 cores.

All compute happens in "transposed" (feature-major) space, which is the
natural layout of the inputs -- x arrives as [C, H*W] per image -- so the
kernel needs zero on-chip transposes:
  qT,kT  : [dk, tok]  = w_q_cols.T @ x         (lhsT = w slice, rhs = x)
  v      : [tok, dk]  = x_tile.T @ w_v_cols
  S^T    : [j, i]     = kT_slice.T @ qT        (K = dk = 64)
  P^T    : exp(S^T/8) on ScalarE, no max-subtraction (|S/8| <~ 6, fp32-safe)
  res^T  : [dk+1, i]  = v_aug.T @ P^T          (ones column -> row 64 = denom)
  out^T  : [c, t]     = w_out_rows.T @ res^T   (+bias +residual fused on DVE)

Matmul operands are bfloat16 with fp32 PSUM accumulation (measured end-to-end
rel err ~4e-4).  Host-side run() pre-gathers the per-kind weight columns,
pre-casts weights/x to bf16, and pre-shapes the biases (q/k/out biases are
per-partition columns in transposed space and ride along the PSUM->SBUF
copies on the DVE; the v bias is added from a partition-broadcast row).
Softmax normalization: reciprocal of the denominator row (DVE), broadcast
across partitions on the otherwise-idle GPSIMD engine, one DVE multiply.

Heads are stored pairwise in 128-partition tiles ([q_{2m}; q_{2m+1}] etc.),
so the K=64 score matmuls contract over partition ranges 0:64 / 64:128 which
stay aligned between lhsT and rhs.  Odd heads' normalized results are
partition-shifted into rows 64:128 of the pair tile with an SBUF->SBUF DMA.

The two images are software-pipelined: attention (ScalarE-bound exp) of
image i overlaps the qkv projections of image i+1 and the output projection
of image i-1 (both TensorE-bound), keeping PE and ACT simultaneously fed.
"""

from contextlib import ExitStack

import ml_dtypes
import numpy as np

import concourse.bass as bass
import concourse.mybir as mybir
import concourse.tile as tile
from concourse import bacc
from concourse.bass_utils import run_bass_kernel_spmd

F32 = mybir.dt.float32
BF16 = mybir.dt.bfloat16

N_CORES = 8
B_LOC = 2            # images per core
C = 512              # channels
NTOK = 1024          # 32*32 spatial positions
NH = 8               # heads
DK = 64              # head dim
NPAIR = 4            # head pairs
CCH = 4              # channel chunks of 128
TCH = 2              # token chunks of 512
SCALE = DK ** -0.5
MODE = "bf16"


def _emit(tc, x_d, xb_d, wq_d, wk_d, wv_d, wo_d, bqp_d, bkp_d, bop_d, bv_d,
          y_d, repeat=1):
    nc = tc.nc
    mdt = BF16
    ADD = mybir.AluOpType.add

    with ExitStack() as ctx:
        cst = ctx.enter_context(tc.tile_pool(name="cst", bufs=1))
        wq_p = ctx.enter_context(tc.tile_pool(name="wq", bufs=CCH))
        wo_p = ctx.enter_context(tc.tile_pool(name="wo", bufs=NPAIR))
        x_p = ctx.enter_context(tc.tile_pool(name="xp", bufs=2 * CCH))
        xb_p = ctx.enter_context(tc.tile_pool(name="xbp", bufs=2 * CCH))
        q_p = ctx.enter_context(tc.tile_pool(name="qp", bufs=2 * NPAIR))
        k_p = ctx.enter_context(tc.tile_pool(name="kp", bufs=2 * NPAIR))
        v_p = ctx.enter_context(tc.tile_pool(name="vp", bufs=2 * NH + 4))
        pt_p = ctx.enter_context(tc.tile_pool(name="ptp", bufs=8))
        r_p = ctx.enter_context(tc.tile_pool(name="rp", bufs=2 * NPAIR))
        t_p = ctx.enter_context(tc.tile_pool(name="tp", bufs=3))
        dn_p = ctx.enter_context(tc.tile_pool(name="dnp", bufs=3))
        rs_p = ctx.enter_context(tc.tile_pool(name="rsp", bufs=4))
        bc_p = ctx.enter_context(tc.tile_pool(name="bcp", bufs=4))
        o_p = ctx.enter_context(tc.tile_pool(name="op", bufs=4))
        ps = ctx.enter_context(tc.tile_pool(name="ps", bufs=4, space="PSUM"))
        ps_big = ctx.enter_context(tc.tile_pool(name="psb", bufs=2, space="PSUM"))

        # ---- constants / biases (host-prepped, single DMAs) ------------
        def em_biases():
            bqp = cst.tile([128, NPAIR], F32, tag="bqp", name="bqp")
            nc.sync.dma_start(out=bqp[:], in_=bqp_d)
            bkp = cst.tile([128, NPAIR], F32, tag="bkp", name="bkp")
            nc.sync.dma_start(out=bkp[:], in_=bkp_d)
            bop = cst.tile([128, CCH], F32, tag="bop", name="bop")
            nc.sync.dma_start(out=bop[:], in_=bop_d)
            bv_st = cst.tile([1, NH * DK], F32, tag="bvst", name="bv_st")
            nc.gpsimd.dma_start(out=bv_st[:], in_=bv_d)
            bv_bc = cst.tile([128, NH * DK], F32, tag="bvbc", name="bv_bc")
            nc.gpsimd.partition_broadcast(bv_bc[:], bv_st[:])
            return bqp, bkp, bop, bv_bc

        # ---- weights (host-prepped bf16, per-kind contiguous) ----------
        w_kind = {"q": [], "k": [], "v": []}
        wo_t = []
        W_DRAM = {"q": wq_d, "k": wk_d, "v": wv_d}

        def em_weights(kind):
            for cc in range(CCH):
                wt = wq_p.tile([128, NH * DK], mdt, tag=f"w{kind}",
                               name=f"w{kind}{cc}", bufs=CCH)
                nc.sync.dma_start(
                    out=wt[:], in_=W_DRAM[kind][cc * 128:(cc + 1) * 128, :])
                w_kind[kind].append(wt)

        def em_wout():
            for m in range(NPAIR):
                wt = wo_p.tile([128, C], mdt, tag="wot", name=f"wot{m}")
                nc.sync.dma_start(out=wt[:],
                                    in_=wo_d[m * 128:(m + 1) * 128, :])
                wo_t.append(wt)

        # ---- software-pipelined per-image stages -----------------------
        n_imgs = B_LOC * repeat
        state = {}

        def em_x(i):
            b = i % B_LOC
            xb_t = []
            for cc in range(CCH):
                xb = xb_p.tile([128, NTOK], mdt, tag="xbt", name=f"xb{i}_{cc}")
                nc.sync.dma_start(out=xb[:],
                                  in_=xb_d[b, cc * 128:(cc + 1) * 128, :])
                xb_t.append(xb)
            state[i] = {"x": [], "xb": xb_t, "q": {}, "k": {}, "v": {},
                        "r": None}

        def em_xres(i):
            # residual fp32 x: only needed at outproj time; keep it off the
            # startup-critical HWDGE queue
            b = i % B_LOC
            for cc in range(CCH):
                xt = x_p.tile([128, NTOK], F32, tag="xt", name=f"xt{i}_{cc}")
                nc.gpsimd.dma_start(out=xt[:],
                                    in_=x_d[b, cc * 128:(cc + 1) * 128, :])
                state[i]["x"].append(xt)

        def em_qkv_unit(i, m, kind, ch):
            # one PSUM group: quarter of a head-pair projection (~0.85us PE)
            st = state[i]
            pool, bcol = (q_p, bqp) if kind == "q" else (k_p, bkp)
            if m not in st[kind]:
                st[kind][m] = pool.tile([128, NTOK], mdt, tag=f"{kind}t",
                                        name=f"{kind}t{i}_{m}")
            dst = st[kind][m]
            ps_qk = ps.tile([128, 512], F32, tag="ps",
                            name=f"psqk{i}_{m}_{kind}_{ch}")
            for cc in range(CCH):
                nc.tensor.matmul(
                    ps_qk[:],
                    w_kind[kind][cc][:, m * 128:(m + 1) * 128],
                    st["xb"][cc][:, ch * 512:(ch + 1) * 512],
                    start=(cc == 0), stop=(cc == CCH - 1))
            # copy out + per-partition bias, fused on DVE
            nc.vector.tensor_scalar_add(
                dst[:, ch * 512:(ch + 1) * 512], ps_qk[:],
                bcol[:, m:m + 1])

        def em_qkv_pair(i, m):
            for kind in ("q", "k"):
                for ch in range(TCH):
                    em_qkv_unit(i, m, kind, ch)

        def em_v(i, tt):
            st = state[i]
            ps_v = ps.tile([128, 512], F32, tag="ps", name=f"psv{i}_{tt}")
            for cc in range(CCH):
                nc.tensor.matmul(
                    ps_v[:],
                    st["xb"][cc][:, tt * 128:(tt + 1) * 128],
                    w_kind["v"][cc][:],
                    start=(cc == 0), stop=(cc == CCH - 1))
            vt = v_p.tile([128, NH * (DK + 1)], mdt, tag="vt",
                          name=f"vt{i}_{tt}")
            vv = vt.rearrange("p (h e) -> p h e", h=NH)
            nc.vector.memset(vv[:, :, DK:DK + 1], 1.0)
            # v bias varies along the free dim: add pre-broadcast rows
            nc.vector.tensor_tensor(
                vv[:, :, 0:DK],
                ps_v.rearrange("p (h d) -> p h d", h=NH),
                bv_bc[:].rearrange("p (h d) -> p h d", h=NH),
                op=ADD)
            st["v"][tt] = vt

        def em_attn_head(i, h):
            st = state[i]
            if st["r"] is None:
                st["r"] = [r_p.tile([128, NTOK], mdt, tag="rt",
                                    name=f"rt{i}_{n}") for n in range(NPAIR)]
            m, hf = divmod(h, 2)
            rs = slice(64 * hf, 64 * hf + 64)
            res_ps = [ps.tile([128, 512], F32, tag="ps",
                              name=f"res{i}_{h}_{n}") for n in range(TCH)]
            for j in range(NH):
                yield
                sps = ps_big.tile([128, NTOK], F32, tag="sps",
                                  name=f"sps{i}_{h}_{j}")
                for ic in range(TCH):
                    nc.tensor.matmul(
                        sps[:, ic * 512:(ic + 1) * 512],
                        st["k"][m][rs, j * 128:(j + 1) * 128],
                        st["q"][m][rs, ic * 512:(ic + 1) * 512],
                        start=True, stop=True)
                pt = pt_p.tile([128, NTOK], mdt, tag="pt",
                               name=f"pt{i}_{h}_{j}")
                nc.scalar.activation(
                    pt[:], sps[:], mybir.ActivationFunctionType.Exp,
                    scale=SCALE)
                for ic in range(TCH):
                    nc.tensor.matmul(
                        res_ps[ic][0:DK + 1, :],
                        st["v"][j][:, h * (DK + 1):(h + 1) * (DK + 1)],
                        pt[:, ic * 512:(ic + 1) * 512],
                        start=(j == 0), stop=(j == NH - 1))
            # normalize: copy res to SBUF first (releases the PSUM slots in
            # ~0.5us instead of holding them through the whole recip ->
            # POOL-broadcast -> multiply chain), then reciprocal of the
            # denom row, GPSIMD partition-broadcast, one DVE multiply
            tmp = (t_p.tile([64, NTOK], mdt, tag="tmp", name=f"tmp{i}_{h}")
                   if hf == 1 else None)
            for ic in range(TCH):
                rsb = rs_p.tile([DK + 1, 512], F32, tag="rsb",
                                name=f"rsb{i}_{h}_{ic}")
                nc.vector.tensor_copy(rsb[:], res_ps[ic][0:DK + 1, :])
                rc = dn_p.tile([1, 512], F32, tag="dn", name=f"rc{i}_{h}_{ic}")
                nc.vector.reciprocal(rc[:], rsb[DK:DK + 1, :])
                bc = bc_p.tile([64, 512], F32, tag="bc",
                               name=f"bc{i}_{h}_{ic}")
                nc.gpsimd.partition_broadcast(bc[:], rc[:])
                dst = (st["r"][m] if hf == 0 else tmp)[0:DK,
                                                       ic * 512:(ic + 1) * 512]
                nc.vector.tensor_tensor(
                    dst, rsb[0:DK, :], bc[:],
                    op=mybir.AluOpType.mult)
            if hf == 1:
                # partition-shift odd head into rows 64:128 of pair tile
                nc.gpsimd.dma_start(out=st["r"][m][64:128, :],
                                    in_=tmp[0:DK, :])

        def em_outproj(i, idx):
            b = i % B_LOC
            st = state[i]
            ct, ch = divmod(idx, TCH)
            ops = ps.tile([128, 512], F32, tag="ps", name=f"pso{i}_{idx}")
            for m in range(NPAIR):
                nc.tensor.matmul(
                    ops[:],
                    wo_t[m][:, ct * 128:(ct + 1) * 128],
                    st["r"][m][:, ch * 512:(ch + 1) * 512],
                    start=(m == 0), stop=(m == NPAIR - 1))
            ot = o_p.tile([128, 512], F32, tag="ot", name=f"ot{i}_{idx}")
            # (psum + b_out) + x, fused on DVE
            nc.vector.scalar_tensor_tensor(
                ot[:], ops[:], bop[:, ct:ct + 1],
                st["x"][ct][:, ch * 512:(ch + 1) * 512],
                op0=ADD, op1=ADD)
            nc.sync.dma_start(
                out=y_d[b, ct * 128:(ct + 1) * 128,
                        ch * 512:(ch + 1) * 512],
                in_=ot[:])


        # image-0 startup: wq streams on the gpsimd queue while xb + wk
        # stream on the sync queue, so the first matmul's two deps arrive
        # in parallel; biases follow on sync, v weights later, wout last
        state[0] = {"x": [], "xb": [], "q": {}, "k": {}, "v": {}, "r": None}
        for cc in range(CCH):
            wt = wq_p.tile([128, NH * DK], mdt, tag="wq",
                           name=f"wq{cc}", bufs=CCH)
            nc.gpsimd.dma_start(out=wt[:], in_=wq_d[cc * 128:(cc + 1) * 128, :])
            w_kind["q"].append(wt)
            xb = xb_p.tile([128, NTOK], mdt, tag="xbt", name=f"xb0_{cc}")
            nc.sync.dma_start(out=xb[:], in_=xb_d[0, cc * 128:(cc + 1) * 128, :])
            state[0]["xb"].append(xb)
        em_weights("k")
        bqp, bkp, bop, bv_bc = em_biases()
        for m in range(NPAIR):
            em_qkv_pair(0, m)
        em_weights("v")
        for tt in range(NH):
            em_v(0, tt)
        em_xres(0)
        em_wout()
        HEAD_ORDER = [1, 0, 3, 2, 5, 4, 7, 6]  # odd first: the odd head's
        # partition-shift DMA runs under the even head's attention, so the
        # pair tile is complete (outproj-ready) right when the pair ends.

        def fill_units(i):
            # fine-grained (~1us PE) fill items for the ACT-bound j-loops
            if i < n_imgs:
                yield lambda: em_x(i)
                for m in range(NPAIR):
                    for kind in ("q", "k"):
                        for ch in range(TCH):
                            yield (lambda m=m, kind=kind, ch=ch:
                                   em_qkv_unit(i, m, kind, ch))
                for tt in range(NH):
                    yield lambda tt=tt: em_v(i, tt)
                yield lambda: em_xres(i)
            if i - 2 >= 0:
                for pidx in range(CCH * TCH):
                    yield lambda pidx=pidx: em_outproj(i - 2, pidx)

        for i in range(n_imgs):
            fill = list(fill_units(i + 1))
            stride = max(2, (NH * NH) // max(1, len(fill)))
            fi = 0
            tick = 0
            for h in HEAD_ORDER:
                for _ in em_attn_head(i, h):
                    tick += 1
                    if tick % stride == 0 and fi < len(fill):
                        fill[fi]()
                        fi += 1
            while fi < len(fill):
                fill[fi]()
                fi += 1
            if i - 2 in state:
                del state[i - 2]
        for pidx in range(CCH * TCH):
            em_outproj(n_imgs - 1, pidx)


def build_program(mode=MODE, repeat=1):
    nc = bacc.Bacc("TRN2", target_bir_lowering=False, debug=False)
    dt = nc.dram_tensor
    x_d = dt("x_loc", [B_LOC, C, NTOK], F32, kind="ExternalInput").ap()
    xb_d = dt("xb_loc", [B_LOC, C, NTOK], BF16, kind="ExternalInput").ap()
    wq_d = dt("wq", [C, NH * DK], BF16, kind="ExternalInput").ap()
    wk_d = dt("wk", [C, NH * DK], BF16, kind="ExternalInput").ap()
    wv_d = dt("wv", [C, NH * DK], BF16, kind="ExternalInput").ap()
    wo_d = dt("wo", [NH * DK, C], BF16, kind="ExternalInput").ap()
    bqp_d = dt("bqp", [128, NPAIR], F32, kind="ExternalInput").ap()
    bkp_d = dt("bkp", [128, NPAIR], F32, kind="ExternalInput").ap()
    bop_d = dt("bop", [128, CCH], F32, kind="ExternalInput").ap()
    bv_d = dt("bv", [1, NH * DK], F32, kind="ExternalInput").ap()
    y_d = dt("y", [B_LOC, C, NTOK], F32, kind="ExternalOutput").ap()
    with tile.TileContext(nc) as tc:
        _emit(tc, x_d, xb_d, wq_d, wk_d, wv_d, wo_d, bqp_d, bkp_d, bop_d,
              bv_d, y_d, repeat=repeat)
    nc.compile()
    return nc


_NC_CACHE = {}


def _get_program(mode=MODE, repeat=1):
    key = (mode, repeat)
    if key not in _NC_CACHE:
        _NC_CACHE[key] = build_program(mode, repeat)
    return _NC_CACHE[key]


def host_prep(inputs):
    """Pre-gather weights per kind, cast to bf16, shape biases."""
    bf16 = ml_dtypes.bfloat16
    x = np.ascontiguousarray(np.asarray(inputs["x"], dtype=np.float32))
    B = x.shape[0]
    xs = x.reshape(B, C, NTOK)
    wp = np.asarray(inputs["w_proj"], dtype=np.float32)
    bp = np.asarray(inputs["b_proj"], dtype=np.float32)
    wo = np.asarray(inputs["w_out"], dtype=np.float32)
    bo = np.asarray(inputs["b_out"], dtype=np.float32)

    w3 = wp.reshape(C, NH, 3, DK)               # [c, h, {q,k,v}, d]
    bp3 = bp.reshape(NH, 3, DK)
    common = {
        "wq": np.ascontiguousarray(
            w3[:, :, 0, :].reshape(C, NH * DK).astype(bf16)),
        "wk": np.ascontiguousarray(
            w3[:, :, 1, :].reshape(C, NH * DK).astype(bf16)),
        "wv": np.ascontiguousarray(
            w3[:, :, 2, :].reshape(C, NH * DK).astype(bf16)),
        "wo": np.ascontiguousarray(wo.astype(bf16)),
        # q/k/out biases as per-partition columns (pair / c-tile layout)
        "bqp": np.ascontiguousarray(
            bp3[:, 0, :].reshape(NPAIR, 128).T.astype(np.float32)),
        "bkp": np.ascontiguousarray(
            bp3[:, 1, :].reshape(NPAIR, 128).T.astype(np.float32)),
        "bop": np.ascontiguousarray(
            bo.reshape(CCH, 128).T.astype(np.float32)),
        "bv": np.ascontiguousarray(
            bp3[:, 2, :].reshape(1, NH * DK).astype(np.float32)),
    }
    xb = xs.astype(bf16)
    return xs, xb, common


def run(inputs, mode=MODE, trace=False, repeat=1):
    """Run on 8 cores; returns (y_full [16,512,32,32] f32, results)."""
    xs, xb, common = host_prep(inputs)
    B = xs.shape[0]
    nc = _get_program(mode, repeat)
    in_maps = []
    for c in range(N_CORES):
        m = {"x_loc": np.ascontiguousarray(xs[c * B_LOC:(c + 1) * B_LOC]),
             "xb_loc": np.ascontiguousarray(xb[c * B_LOC:(c + 1) * B_LOC])}
        m.update(common)
        in_maps.append(m)
    res = run_bass_kernel_spmd(nc, in_maps, core_ids=list(range(N_CORES)),
                               trace=trace)
    y = np.concatenate([res.results[c]["y"] for c in range(N_CORES)], axis=0)
    return y.reshape(B, C, 32, 32), res


def kernel(**inputs):
    y, _ = run(inputs)
    return y


if __name__ == "__main__":
    nc = build_program()
    print("program built + compiled OK")
